# revision 23
# baseline (speedup 1.0000x reference)
"""Distributed Bass kernel: fused multi-head attention block on 8 TRN2 NeuronCores.

Problem: x[2,2048,1024] -> QKV proj -> RoPE(q,k) -> softmax(q k^T/8) v -> out proj.

Sharding: tensor-parallel over heads. 16 heads / 8 cores = 2 heads per core.
Each core computes QKV for its 2 heads (full sequence), RoPE, attention, then
an AllToAll converts head-sharding to token-sharding (each core ends with 512
of the 4096 tokens, all 1024 features) so the output projection runs against
the FULL Wout with no AllReduce. Host concatenates the 8 token slices.

All layouts are pre-arranged on the host so the device never transposes x:
 - xT      [1024, 4096]  x^T               (shared by all cores)
 - wqkvT   [1024, 384]   [qA qB kA kB vA vB] rows of Wqkv, transposed (per core)
 - woutT   [1024, 1024]  Wout^T            (shared)
 - cos2/sin2 [128, 4096] RoPE factors expanded to d-major, two head copies
 - perm    [128, 128]    pair-swap permutation (rope partner via PE matmul)
 - ident   [128, 128]    identity for PE transposes

Compute dtype bf16 (PE 1 cycle/row), f32 PSUM accumulation. Softmax skips the
max-subtraction (scores ~N(0,2), |s|<~12, exp safe in f32) and folds the
denominator into the PV matmul via a ones-column appended to v.
"""

import numpy as np
import ml_dtypes

B, N, HID = 2, 2048, 1024
H, DH = 16, 64
NCORES = 8
HPC = H // NCORES          # heads per core = 2
T = B * N                  # 4096 flattened tokens
TS = T // NCORES           # 512 tokens per core after AllToAll
EPC = HPC * DH             # 128 features per core
CH = 512                   # token chunk for QKV phase
NCH = T // CH              # 8 chunks
KT = 128                   # key tile
QC = 512                   # query chunk in attention
VSLOT = 2 * (DH + 1)       # 130: [vA(64) | oneA | vB(64) | oneB]

_bf16 = ml_dtypes.bfloat16


def _build_graph():
    import concourse.bass as bass
    import concourse.mybir as mybir
    import concourse.tile as tile
    from concourse import bacc

    f32 = mybir.dt.float32
    bf16 = mybir.dt.bfloat16

    nc = bacc.Bacc("TRN2", target_bir_lowering=False, debug=False, num_devices=NCORES)

    xT_e = nc.declare_dram_parameter("xT", [HID, T], bf16, isOutput=False)
    wqkvT_e = nc.declare_dram_parameter("wqkvT", [HID, 3 * EPC], bf16, isOutput=False)
    woutT_e = nc.declare_dram_parameter("woutT", [HID, HID], bf16, isOutput=False)
    cos2_e = nc.declare_dram_parameter("cos2", [2 * DH, T], bf16, isOutput=False)
    sin2_e = nc.declare_dram_parameter("sin2", [2 * DH, T], bf16, isOutput=False)
    perm_e = nc.declare_dram_parameter("perm", [128, 128], bf16, isOutput=False)
    ident_e = nc.declare_dram_parameter("ident", [128, 128], bf16, isOutput=False)
    out_e = nc.declare_dram_parameter("out", [TS, HID], f32, isOutput=True)

    with tile.TileContext(nc) as tc:
        with (
            tc.tile_pool(name="const", bufs=1) as cpool,
            tc.tile_pool(name="work", bufs=1) as wpool,
            tc.tile_pool(name="stream", bufs=4) as spool,
            tc.tile_pool(name="psum", bufs=2, space="PSUM") as pspool,
            tc.tile_pool(name="dram", bufs=1, space="DRAM") as dpool,
        ):
            # ---- constants / weights ----
            wqkvT = cpool.tile([128, 8 * 3 * EPC], bf16)       # 8 k-tiles side by side
            for kt in range(8):
                nc.gpsimd.dma_start(
                    wqkvT[:, kt * 3 * EPC:(kt + 1) * 3 * EPC],
                    wqkvT_e[kt * 128:(kt + 1) * 128, :],
                )
            woutT = cpool.tile([128, 8 * HID], bf16)
            cos2 = cpool.tile([128, T], bf16)
            sin2 = cpool.tile([128, T], bf16)
            perm = cpool.tile([128, 128], bf16)
            nc.gpsimd.dma_start(perm[:, :], perm_e[:, :])
            ident = cpool.tile([128, 128], bf16)
            nc.gpsimd.dma_start(ident[:, :], ident_e[:, :])

            # ---- persistent working tensors ----
            # HAM note: the PE clock-gate only releases (2.4 GHz) for
            # full-geometry matmuls. Scores therefore contract over the full
            # 128 partitions (both heads' d stacked) against ZERO-PADDED
            # per-head q copies, and PV uses a 128-wide per-head v table
            # [v(64) | ones(1) | zeros(63)] so every matmul is 128x128xN.
            q_sb = wpool.tile([128, T], bf16)      # raw q (rope intermediate)
            k_sb = wpool.tile([128, T], bf16)      # becomes roped k
            qzA = wpool.tile([128, T], bf16)       # roped qA rows 0-63, 0 below
            qzB = wpool.tile([128, T], bf16)       # roped qB rows 64-127, 0 above
            vT_sb = wpool.tile([128, T], bf16)     # v transposed [e, t]
            vexA = wpool.tile([128, 32 * 128], bf16)  # head-A v table per slot
            vexB = wpool.tile([128, 32 * 128], bf16)
            ovT = wpool.tile([128, T], bf16)       # attention out ^T
            gT = wpool.tile([128, 8 * TS], bf16)   # post-A2A gathered [e, t]

            nc.vector.memset(qzA[DH:128, :], 0.0)
            nc.vector.memset(qzB[0:DH, :], 0.0)
            vexA3 = vexA.rearrange("p (s c) -> p s c", c=128)
            vexB3 = vexB.rearrange("p (s c) -> p s c", c=128)
            nc.vector.memset(vexA3[:, :, DH:128], 0.0)
            nc.vector.memset(vexB3[:, :, DH:128], 0.0)
            nc.vector.memset(vexA3[:, :, DH:DH + 1], 1.0)
            nc.vector.memset(vexB3[:, :, DH:DH + 1], 1.0)

            # ================= Phase 1: QKV + RoPE + v-transpose =================
            for c in range(NCH):
                xs = []
                for kt in range(8):
                    xt = spool.tile([128, CH], bf16, tag="xs", bufs=10)
                    nc.gpsimd.dma_start(
                        xt[:, :], xT_e[kt * 128:(kt + 1) * 128, c * CH:(c + 1) * CH]
                    )
                    xs.append(xt)
                if c == 0:
                    nc.gpsimd.dma_start(cos2[:, :], cos2_e[:, :])
                    nc.gpsimd.dma_start(sin2[:, :], sin2_e[:, :])
                for which, dest in ((0, q_sb), (1, k_sb), (2, vT_sb)):
                    ps = pspool.tile([128, CH], f32, tag="mm", bufs=1)
                    for kt in range(8):
                        nc.tensor.matmul(
                            ps[:, :],
                            wqkvT[:, kt * 3 * EPC + which * EPC:
                                  kt * 3 * EPC + (which + 1) * EPC],
                            xs[kt][:, :],
                            start=(kt == 0),
                            stop=(kt == 7),
                        )
                    nc.vector.tensor_copy(dest[:, c * CH:(c + 1) * CH], ps[:, :])

                # RoPE on q and k: t = P@x * sin2 ; rot = x*cos2 + t
                # k rotates in place; q writes split per head into qzA/qzB.
                sl = slice(c * CH, (c + 1) * CH)
                for srd, dests in ((q_sb, ((qzA, 0, DH), (qzB, DH, 128))),
                                   (k_sb, ((k_sb, 0, 128),))):
                    pps = pspool.tile([128, CH], f32, tag="aux", bufs=3)
                    nc.tensor.matmul(
                        pps[:, :], perm[:, :], srd[:, sl],
                        start=True, stop=True,
                    )
                    tmp = spool.tile([128, CH], bf16, tag="ropetmp", bufs=2)
                    nc.vector.tensor_mul(tmp[:, :], pps[:, :], sin2[:, sl])
                    nc.vector.tensor_mul(srd[:, sl], srd[:, sl], cos2[:, sl])
                    for dst, p0, p1 in dests:
                        nc.vector.tensor_add(
                            dst[p0:p1, sl], srd[p0:p1, sl], tmp[p0:p1, :]
                        )

                # transpose v chunk into per-head 128-wide v tables
                for tt in range(CH // 128):
                    slot = c * (CH // 128) + tt
                    tp = pspool.tile([128, 128], bf16, tag="sc", bufs=2)
                    nc.tensor.transpose(
                        tp[:, :],
                        vT_sb[:, c * CH + tt * 128:c * CH + (tt + 1) * 128],
                        ident[:, :],
                    )
                    nc.vector.tensor_copy(
                        vexA3[:, slot, 0:DH], tp[:, 0:DH]
                    )
                    nc.vector.tensor_copy(
                        vexB3[:, slot, 0:DH], tp[:, DH:2 * DH]
                    )

            # ================= Phase 2: attention per (batch, head) =================
            # Zippered software pipeline: the scores+exp of query-chunk i are
            # interleaved pairwise with the PV matmuls of chunk i-1 so the PE
            # stream stays dense (HAM stays at full clock) while ACT churns
            # through the exps. exp runs on double-width (1024) PSUM tiles to
            # amortize its fixed cost.
            NKT = N // KT                      # 16 key tiles per chunk
            vtabs = (vexA, vexB)

            def emit_pv_pair(st, pair):
                (b, qc, opsAB, expTs) = st
                for h in range(HPC):
                    for kt in (2 * pair, 2 * pair + 1):
                        slot = b * (N // 128) + kt
                        nc.tensor.matmul(
                            opsAB[h][:, :],
                            vtabs[h][:, slot * 128:(slot + 1) * 128],
                            expTs[h][:, kt * QC:(kt + 1) * QC],
                            start=(kt == 0),
                            stop=(kt == NKT - 1),
                        )

            def emit_normalize(st):
                (b, qc, opsAB, expTs) = st
                q0 = b * N + qc * QC
                for h in range(HPC):
                    hr = h * DH
                    ops = opsAB[h]
                    den = spool.tile([1, QC], f32, tag="den", bufs=2)
                    nc.vector.tensor_copy(den[0:1, :], ops[DH:DH + 1, :])
                    rec = spool.tile([1, QC], f32, tag="rec", bufs=2)
                    nc.vector.reciprocal_approx_fast(rec[0:1, :], den[0:1, :])
                    bcs = spool.tile([64, QC], f32, tag="bcs", bufs=2)
                    nc.gpsimd.partition_broadcast(bcs[:, :], rec[0:1, :])
                    nc.vector.tensor_mul(
                        ovT[hr:hr + DH, q0:q0 + QC], ops[0:DH, :], bcs[:, :]
                    )

            # Split AllToAll: the batch-0 half fires as soon as batch-0
            # attention is done and runs under batch-1 compute; unused chunks
            # are zero so SPMD-uniform gT = a2a1_out + a2a2_out reassembles.
            a2a_in1 = dpool.tile([NCORES * 128, TS], bf16)
            a2a_out1 = dpool.tile([NCORES * 128, TS], bf16)
            a2a_in2 = dpool.tile([NCORES * 128, TS], bf16)
            a2a_out2 = dpool.tile([NCORES * 128, TS], bf16)
            zero_sb = wpool.tile([128, TS], bf16)
            nc.vector.memset(zero_sb[:, :], 0.0)
            for j in range(NCORES // 2):
                nc.gpsimd.dma_start(
                    a2a_in1[(j + 4) * 128:(j + 5) * 128, :], zero_sb[:, :]
                )
                nc.gpsimd.dma_start(
                    a2a_in2[j * 128:(j + 1) * 128, :], zero_sb[:, :]
                )

            def emit_a2a(buf_in, buf_out, jlist):
                for j in jlist:
                    nc.gpsimd.dma_start(
                        buf_in[j * 128:(j + 1) * 128, :],
                        ovT[:, j * TS:(j + 1) * TS],
                    )
                nc.gpsimd.collective_compute(
                    "AllToAll",
                    mybir.AluOpType.bypass,
                    ins=[buf_in.opt()],
                    outs=[buf_out.opt()],
                    replica_groups=[list(range(NCORES))],
                )

            for kt in range(8):
                nc.gpsimd.dma_start(
                    woutT[:, kt * HID:(kt + 1) * HID],
                    woutT_e[kt * 128:(kt + 1) * 128, :],
                )

            qzs = (qzA, qzB)
            pending = None
            for b in range(B):
                for qc in range(N // QC):
                    q0 = b * N + qc * QC
                    expTs = (spool.tile([128, NKT * QC], bf16, name="expTA",
                                        tag="expTA", bufs=2),
                             spool.tile([128, NKT * QC], bf16, name="expTB",
                                        tag="expTB", bufs=2))
                    for pair in range(NKT // 2):
                        for h in range(HPC):
                            sps = pspool.tile([128, 2 * QC], f32, tag="sc",
                                              bufs=2)
                            for half in range(2):
                                k0 = b * N + (2 * pair + half) * KT
                                nc.tensor.matmul(
                                    sps[:, half * QC:(half + 1) * QC],
                                    k_sb[:, k0:k0 + KT],
                                    qzs[h][:, q0:q0 + QC],
                                    start=True, stop=True,
                                )
                            nc.scalar.activation(
                                expTs[h][:, 2 * pair * QC:(2 * pair + 2) * QC],
                                sps[:, :],
                                mybir.ActivationFunctionType.Exp,
                                scale=DH ** -0.5,
                            )
                        if pending is not None:
                            emit_pv_pair(pending, pair)
                            if pair == NKT // 2 - 1:
                                emit_normalize(pending)
                                if pending[0] == 0 and pending[1] == N // QC - 1:
                                    emit_a2a(a2a_in1, a2a_out1,
                                             range(NCORES // 2))
                    opsAB = (pspool.tile([128, QC], f32, name="opsA",
                                         tag="aux", bufs=3),
                             pspool.tile([128, QC], f32, name="opsB",
                                         tag="aux", bufs=3))
                    pending = (b, qc, opsAB, expTs)
            for pair in range(NKT // 2):
                emit_pv_pair(pending, pair)
            emit_normalize(pending)
            emit_a2a(a2a_in2, a2a_out2, range(NCORES // 2, NCORES))

            # gT = a2a_out1 + a2a_out2 (one of the two is zeros per block)
            for et in range(NCORES):
                g1 = spool.tile([128, TS], bf16, tag="g1", bufs=2)
                nc.gpsimd.dma_start(
                    g1[:, :], a2a_out1[et * 128:(et + 1) * 128, :]
                )
                g2 = spool.tile([128, TS], bf16, tag="g2", bufs=2)
                nc.gpsimd.dma_start(
                    g2[:, :], a2a_out2[et * 128:(et + 1) * 128, :]
                )
                nc.vector.tensor_add(
                    gT[:, et * TS:(et + 1) * TS], g1[:, :], g2[:, :]
                )

            # ================= Phase 4: output projection =========================
            for m in range(TS // 128):
                for nn in range(HID // 512):
                    odps = pspool.tile([128, 512], f32, tag="mm", bufs=1)
                    for et in range(8):
                        nc.tensor.matmul(
                            odps[:, :],
                            gT[:, et * TS + m * 128:et * TS + (m + 1) * 128],
                            woutT[:, et * HID + nn * 512:et * HID + (nn + 1) * 512],
                            start=(et == 0),
                            stop=(et == 7),
                        )
                    osb = spool.tile([128, 512], f32, tag="osb", bufs=2)
                    nc.vector.tensor_copy(osb[:, :], odps[:, :])
                    nc.gpsimd.dma_start(
                        out_e[m * 128:(m + 1) * 128, nn * 512:(nn + 1) * 512],
                        osb[:, :],
                    )

    nc.finalize()
    return nc


def _host_inputs(x, rope, Wqkv, Wout):
    """Build the 8 per-core input maps with host-side layout prep."""
    xf = np.ascontiguousarray(x.reshape(T, HID).T).astype(_bf16)        # [1024, 4096]
    woutT = np.ascontiguousarray(Wout.T).astype(_bf16)                  # [1024, 1024]

    rf = rope.reshape(T, DH)                                            # [4096, 64]
    cosE = np.repeat(rf[:, 0::2], 2, axis=1).T                          # [64, 4096]
    sinE = np.repeat(rf[:, 1::2], 2, axis=1).T
    sgn = np.where(np.arange(DH) % 2 == 0, -1.0, 1.0)[:, None]
    sinS = (sinE * sgn)
    cos2 = np.ascontiguousarray(np.concatenate([cosE, cosE], 0)).astype(_bf16)
    sin2 = np.ascontiguousarray(np.concatenate([sinS, sinS], 0)).astype(_bf16)

    pm = np.zeros((128, 128), np.float32)
    for d in range(128):
        pm[d ^ 1, d] = 1.0       # partner[d] = q[d^1]; lhsT = S (symmetric)
    perm = pm.astype(_bf16)
    ident = np.eye(128, dtype=np.float32).astype(_bf16)

    w3 = Wqkv.reshape(3, H, DH, HID)
    in_maps = []
    for c in range(NCORES):
        blocks = []
        for which in range(3):
            for hl in range(HPC):
                blocks.append(w3[which, 2 * c + hl])                    # [64, 1024]
        wq = np.concatenate(blocks, 0)                                  # [384, 1024]
        wqkvT = np.ascontiguousarray(wq.T).astype(_bf16)                # [1024, 384]
        in_maps.append({
            "xT": xf, "wqkvT": wqkvT, "woutT": woutT,
            "cos2": cos2, "sin2": sin2, "perm": perm, "ident": ident,
        })
    return in_maps


_CACHE = {}


def kernel(x, rope, Wqkv, Wout):
    from concourse.bass_utils import run_bass_kernel_spmd

    if "nc" not in _CACHE:
        _CACHE["nc"] = _build_graph()
    nc = _CACHE["nc"]
    in_maps = _host_inputs(np.asarray(x, np.float32), np.asarray(rope, np.float32),
                           np.asarray(Wqkv, np.float32), np.asarray(Wout, np.float32))
    res = run_bass_kernel_spmd(nc, in_maps, core_ids=list(range(NCORES)))
    parts = [np.asarray(res.results[i]["out"], np.float32) for i in range(NCORES)]
    full = np.concatenate(parts, 0)                                     # [4096, 1024]
    return full.reshape(B, N, HID)
